# revision 13
# baseline (speedup 1.0000x reference)
"""Trainium2 Bass kernel: MeanFieldMultiDimensionalLogisticRegression.

Computes, for X:[N,D], z:[S], w_mu:[D], w_log_var:[D]:
    mean_i = X @ w_mu                       [N]
    var_i  = sum(X^2 * exp(w_log_var), -1)  [N]
    act    = std_i[:,None]*z[None,:] + mean_i[:,None]   [N,S]
    Y      = sigmoid(act)
returns (Y, act).

Data-parallel over 8 NeuronCores: X and outputs sharded along N;
w_mu / w_log_var / z replicated.

Per-core device program (2048 rows = 16 tiles of 128 rows, grouped in
pairs for software pipelining):
  per tile:  DMA X tile (f32)
             DVE scalar_tensor_tensor -> mean_t (fused mult+rowsum, f32)
             DVE mult -> xs = X*sqrt(exp(w_log_var)), written as bf16
             ACT Square(+row-accum) -> var_t
  per group of 2 tiles:
             DVE-only rsqrt (Quake bitcast + 2 Newton steps) -> std
             ACT Identity(zb, scale=std, bias=mean) -> act tile
             ACT Sigmoid(zb, scale=std, bias=mean)  -> Y tile
             DMA store both
The scalar engine stays in one activation-table set (square/identity/
sigmoid all live in sigmoid_and_others), so there are no ~2.7us table
switches and no global sqrt barrier.
"""

import os
import numpy as np

import concourse.bass as bass
import concourse.tile as tile
from concourse import bacc, mybir
from concourse.bass_utils import run_bass_kernel_spmd

N, D, S = 16384, 1024, 256
NCORES = 8
NSHARD = N // NCORES  # 2048 rows per core
P = 128               # SBUF partitions
NT = NSHARD // P      # 16 row-tiles per core
G = 1                 # row-tiles per rsqrt/output group
F32 = mybir.dt.float32
BF16 = mybir.dt.bfloat16
I32 = mybir.dt.int32
RSQRT_MAGIC = 0x5F3759DF

_cached_nc = None
last_result = None  # BassKernelResults of the most recent run (for harness)


def build_program(reps=1, xs_bf16=True):
    """Build the per-core Bass/Tile program (identical on all 8 cores).

    reps>1 wraps the computation in an on-device For_i loop -- used only
    for benchmarking (wall-clock slope vs reps)."""
    nc = bacc.Bacc("TRN2", debug=False, num_devices=NCORES)

    x_h = nc.declare_dram_parameter("x", [NSHARD, D], F32, isOutput=False)
    wb_h = nc.declare_dram_parameter("wb", [P, D], F32, isOutput=False)
    qv_h = nc.declare_dram_parameter("qv", [1, D], F32, isOutput=False)
    zv_h = nc.declare_dram_parameter("zv", [1, S], F32, isOutput=False)
    act_h = nc.declare_dram_parameter("act", [NSHARD, S], F32, isOutput=True)
    y_h = nc.declare_dram_parameter("y", [NSHARD, S], F32, isOutput=True)

    AF = mybir.ActivationFunctionType
    OP = mybir.AluOpType
    XSDT = BF16 if xs_bf16 else F32

    with tile.TileContext(nc) as tc:
        with (
            tc.tile_pool(name="consts", bufs=1) as consts,
            tc.tile_pool(name="xp", bufs=5) as xp,
            tc.tile_pool(name="xsp", bufs=3) as xsp,
            tc.tile_pool(name="stats", bufs=1) as stats,
            tc.tile_pool(name="outp", bufs=6) as outp,
        ):
            # w_mu arrives pre-broadcast from the host (512KB) so the first
            # DVE op doesn't wait on an on-device broadcast chain.
            wb = consts.tile([P, D], F32)
            nc.sync.dma_start(out=wb[:], in_=wb_h[:])
            qv = consts.tile([1, D], F32)
            nc.sync.dma_start(out=qv[:], in_=qv_h[:])
            zv = consts.tile([1, S], F32)
            nc.sync.dma_start(out=zv[:], in_=zv_h[:])
            qb = consts.tile([P, D], F32)  # sqrt(exp(w_log_var)) broadcast
            nc.gpsimd.partition_broadcast(qb[:], qv[0:1, :])
            zb = consts.tile([P, S], F32)  # z broadcast
            nc.gpsimd.partition_broadcast(zb[:], zv[0:1, :])

            mean_all = stats.tile([P, NT], F32)
            var_all = stats.tile([P, NT], F32)
            std_all = stats.tile([P, NT], F32)
            rsq_i = stats.tile([P, NT], I32)
            rsq_r = stats.tile([P, NT], F32)
            rsq_a = stats.tile([P, NT], F32)

            def dve_std(cols):
                """std = sqrt(var) on the vector engine only (Quake initial
                guess + 2 Newton steps; rel err ~4e-6), so the scalar engine
                never switches activation-table sets."""
                v = var_all[:, cols]
                nc.vector.tensor_scalar(
                    out=rsq_i[:, cols], in0=v.bitcast(I32), scalar1=1,
                    scalar2=None, op0=OP.logical_shift_right)
                nc.vector.tensor_scalar(
                    out=rsq_i[:, cols], in0=rsq_i[:, cols], scalar1=0,
                    scalar2=None, op0=OP.bitwise_not)
                nc.vector.tensor_scalar(
                    out=rsq_i[:, cols], in0=rsq_i[:, cols],
                    scalar1=RSQRT_MAGIC + 1, scalar2=None, op0=OP.add)
                nc.vector.tensor_copy(rsq_r[:, cols], rsq_i[:, cols].bitcast(F32))
                for _ in range(2):
                    # r = r * (1.5 - 0.5*v*r*r)
                    nc.vector.tensor_mul(rsq_a[:, cols], rsq_r[:, cols],
                                         rsq_r[:, cols])
                    nc.vector.tensor_mul(rsq_a[:, cols], rsq_a[:, cols], v)
                    nc.vector.tensor_scalar(
                        out=rsq_a[:, cols], in0=rsq_a[:, cols], scalar1=-0.5,
                        scalar2=1.5, op0=OP.mult, op1=OP.add)
                    nc.vector.tensor_mul(rsq_r[:, cols], rsq_r[:, cols],
                                         rsq_a[:, cols])
                nc.vector.tensor_mul(std_all[:, cols], v, rsq_r[:, cols])

            def tile_out(t):
                s1 = std_all[:, t:t + 1]
                s2 = mean_all[:, t:t + 1]
                at = outp.tile([P, S], F32)
                nc.scalar.activation(at[:], zb[:], AF.Identity,
                                     bias=s2, scale=s1)
                yt = outp.tile([P, S], F32)
                nc.scalar.activation(yt[:], zb[:], AF.Sigmoid,
                                     bias=s2, scale=s1)
                nc.sync.dma_start(out=act_h[t * P:(t + 1) * P, :], in_=at[:])
                nc.sync.dma_start(out=y_h[t * P:(t + 1) * P, :], in_=yt[:])

            def body():
                for t in range(NT):
                    xt = xp.tile([P, D], F32)
                    nc.sync.dma_start(out=xt[:], in_=x_h[t * P:(t + 1) * P, :])
                    # mean_t = rowsum(X*w_mu) fused on DVE; `out` is scratch
                    xs = xsp.tile([P, D], XSDT)
                    scr = xsp.tile([P, D], F32, tag="scr")
                    nc.vector.scalar_tensor_tensor(
                        out=scr[:], in0=xt[:], scalar=1.0, in1=wb[:],
                        op0=OP.mult, op1=OP.mult,
                        accum_out=mean_all[:, t:t + 1])
                    # xs = X * sqrt(exp(w_log_var)); written bf16 so the
                    # scalar engine can square-accumulate at 2x
                    nc.vector.tensor_mul(xs[:], xt[:], qb[:])
                    # var_t = rowsum(xs^2)
                    sq = xsp.tile([P, D], XSDT, tag="sq")
                    nc.scalar.activation(sq[:], xs[:], AF.Square,
                                         accum_out=var_all[:, t:t + 1])
                    if t % G == G - 1:
                        g = t // G
                        dve_std(slice(g * G, (g + 1) * G))
                        for tt in range(g * G, (g + 1) * G):
                            tile_out(tt)

            if reps == 1:
                body()
            else:
                with tc.For_i(0, reps, 1):
                    body()

    nc.compile()
    return nc


def _get_nc():
    global _cached_nc
    if _cached_nc is None:
        _cached_nc = build_program()
    return _cached_nc


def make_host_inputs(X, z, w_mu, w_log_var):
    """Host-side prep: exp of the [D] vector + broadcast of w_mu."""
    X = np.ascontiguousarray(np.asarray(X, dtype=np.float32))
    z = np.asarray(z, dtype=np.float32)
    w_mu = np.asarray(w_mu, dtype=np.float32)
    w_log_var = np.asarray(w_log_var, dtype=np.float32)
    sqew = np.exp(0.5 * w_log_var).astype(np.float32)  # sqrt(exp(w_log_var))
    wb = np.ascontiguousarray(np.broadcast_to(w_mu, (P, D)))
    qv = np.ascontiguousarray(sqew.reshape(1, D))
    zv = np.ascontiguousarray(z.reshape(1, S))
    in_maps = [
        {"x": X[k * NSHARD:(k + 1) * NSHARD], "wb": wb, "qv": qv, "zv": zv}
        for k in range(NCORES)
    ]
    return in_maps


def kernel(X, z, w_mu, w_log_var):
    global last_result
    nc = _get_nc()
    in_maps = make_host_inputs(X, z, w_mu, w_log_var)
    trace = bool(int(os.environ.get("KTRACE", "0")))
    res = run_bass_kernel_spmd(nc, in_maps, list(range(NCORES)), trace=trace)
    last_result = res
    Y = np.concatenate([r["y"] for r in res.results], axis=0)
    act = np.concatenate([r["act"] for r in res.results], axis=0)
    return (Y, act)


# revision 15
# speedup vs baseline: 1.0845x; 1.0845x over previous
"""Trainium2 Bass kernel: MeanFieldMultiDimensionalLogisticRegression.

Computes, for X:[N,D], z:[S], w_mu:[D], w_log_var:[D]:
    mean_i = X @ w_mu                       [N]
    var_i  = sum(X^2 * exp(w_log_var), -1)  [N]
    act    = std_i[:,None]*z[None,:] + mean_i[:,None]   [N,S]
    Y      = sigmoid(act)
returns (Y, act).

Data-parallel over 8 NeuronCores: X and outputs sharded along N;
w_mu / w_log_var / z replicated.

Per-core device program (2048 rows = 16 tiles of 128 rows, grouped in
pairs for software pipelining):
  per tile:  DMA X tile (f32)
             DVE scalar_tensor_tensor -> mean_t (fused mult+rowsum, f32)
             DVE mult -> xs = X*sqrt(exp(w_log_var)), written as bf16
             ACT Square(+row-accum) -> var_t
  per group of 2 tiles:
             DVE-only rsqrt (Quake bitcast + 2 Newton steps) -> std
             ACT Identity(zb, scale=std, bias=mean) -> act tile
             ACT Sigmoid(zb, scale=std, bias=mean)  -> Y tile
             DMA store both
The scalar engine stays in one activation-table set (square/identity/
sigmoid all live in sigmoid_and_others), so there are no ~2.7us table
switches and no global sqrt barrier.
"""

import os
import numpy as np

import concourse.bass as bass
import concourse.tile as tile
from concourse import bacc, mybir
from concourse.bass_utils import run_bass_kernel_spmd

N, D, S = 16384, 1024, 256
NCORES = 8
NSHARD = N // NCORES  # 2048 rows per core
P = 128               # SBUF partitions
NT = NSHARD // P      # 16 row-tiles per core
G = 2                 # row-tiles per rsqrt/output group
F32 = mybir.dt.float32
BF16 = mybir.dt.bfloat16
I32 = mybir.dt.int32
RSQRT_MAGIC = 0x5F3759DF

_cached_nc = None
last_result = None  # BassKernelResults of the most recent run (for harness)


def build_program(reps=1, xs_bf16=True):
    """Build the per-core Bass/Tile program (identical on all 8 cores).

    reps>1 wraps the computation in an on-device For_i loop -- used only
    for benchmarking (wall-clock slope vs reps)."""
    nc = bacc.Bacc("TRN2", debug=False, num_devices=NCORES)

    x_h = nc.declare_dram_parameter("x", [NSHARD, D], F32, isOutput=False)
    wb_h = nc.declare_dram_parameter("wb", [P, D], F32, isOutput=False)
    qv_h = nc.declare_dram_parameter("qv", [1, D], F32, isOutput=False)
    zv_h = nc.declare_dram_parameter("zv", [1, S], F32, isOutput=False)
    act_h = nc.declare_dram_parameter("act", [NSHARD, S], F32, isOutput=True)
    y_h = nc.declare_dram_parameter("y", [NSHARD, S], F32, isOutput=True)

    AF = mybir.ActivationFunctionType
    OP = mybir.AluOpType
    XSDT = BF16 if xs_bf16 else F32

    with tile.TileContext(nc) as tc:
        with (
            tc.tile_pool(name="consts", bufs=1) as consts,
            tc.tile_pool(name="xp", bufs=5) as xp,
            tc.tile_pool(name="xsp", bufs=3) as xsp,
            tc.tile_pool(name="stats", bufs=1) as stats,
            tc.tile_pool(name="outp", bufs=6) as outp,
        ):
            # w_mu arrives pre-broadcast from the host (512KB) so the first
            # DVE op doesn't wait on an on-device broadcast chain.
            wb = consts.tile([P, D], F32)
            nc.sync.dma_start(out=wb[:], in_=wb_h[:])
            qv = consts.tile([1, D], F32)
            nc.sync.dma_start(out=qv[:], in_=qv_h[:])
            zv = consts.tile([1, S], F32)
            nc.sync.dma_start(out=zv[:], in_=zv_h[:])
            qb = consts.tile([P, D], F32)  # sqrt(exp(w_log_var)) broadcast
            nc.gpsimd.partition_broadcast(qb[:], qv[0:1, :])
            zb = consts.tile([P, S], F32)  # z broadcast
            nc.gpsimd.partition_broadcast(zb[:], zv[0:1, :])

            mean_all = stats.tile([P, NT], F32)
            var_all = stats.tile([P, NT], F32)
            std_all = stats.tile([P, NT], F32)
            rsq_i = stats.tile([P, NT], I32)
            rsq_r = stats.tile([P, NT], F32)
            rsq_a = stats.tile([P, NT], F32)

            def dve_std(cols):
                """std = sqrt(var) on the vector engine only (Quake initial
                guess + 2 Newton steps; rel err ~4e-6), so the scalar engine
                never switches activation-table sets."""
                v = var_all[:, cols]
                nc.vector.tensor_scalar(
                    out=rsq_i[:, cols], in0=v.bitcast(I32), scalar1=1,
                    scalar2=None, op0=OP.logical_shift_right)
                nc.vector.tensor_scalar(
                    out=rsq_i[:, cols], in0=rsq_i[:, cols], scalar1=0,
                    scalar2=None, op0=OP.bitwise_not)
                nc.vector.tensor_scalar(
                    out=rsq_i[:, cols], in0=rsq_i[:, cols],
                    scalar1=RSQRT_MAGIC + 1, scalar2=None, op0=OP.add)
                r0 = rsq_i[:, cols].bitcast(F32)
                nc.vector.tensor_mul(rsq_a[:, cols], r0, r0)
                first = True
                for _ in range(2):
                    # r = r * (1.5 - 0.5*v*r*r)
                    if not first:
                        nc.vector.tensor_mul(rsq_a[:, cols], rsq_r[:, cols],
                                             rsq_r[:, cols])
                    nc.vector.tensor_mul(rsq_a[:, cols], rsq_a[:, cols], v)
                    nc.vector.tensor_scalar(
                        out=rsq_a[:, cols], in0=rsq_a[:, cols], scalar1=-0.5,
                        scalar2=1.5, op0=OP.mult, op1=OP.add)
                    src_r = r0 if first else rsq_r[:, cols]
                    nc.vector.tensor_mul(rsq_r[:, cols], src_r,
                                         rsq_a[:, cols])
                    first = False
                nc.vector.tensor_mul(std_all[:, cols], v, rsq_r[:, cols])

            def tile_out(t):
                s1 = std_all[:, t:t + 1]
                s2 = mean_all[:, t:t + 1]
                at = outp.tile([P, S], F32)
                nc.scalar.activation(at[:], zb[:], AF.Identity,
                                     bias=s2, scale=s1)
                yt = outp.tile([P, S], F32)
                nc.scalar.activation(yt[:], zb[:], AF.Sigmoid,
                                     bias=s2, scale=s1)
                nc.sync.dma_start(out=act_h[t * P:(t + 1) * P, :], in_=at[:])
                nc.sync.dma_start(out=y_h[t * P:(t + 1) * P, :], in_=yt[:])

            def body():
                for t in range(NT):
                    xt = xp.tile([P, D], F32)
                    nc.sync.dma_start(out=xt[:], in_=x_h[t * P:(t + 1) * P, :])
                    # mean_t = rowsum(X*w_mu) fused on DVE; `out` is scratch
                    xs = xsp.tile([P, D], XSDT)
                    scr = xsp.tile([P, D], F32, tag="scr")
                    nc.vector.scalar_tensor_tensor(
                        out=scr[:], in0=xt[:], scalar=1.0, in1=wb[:],
                        op0=OP.mult, op1=OP.mult,
                        accum_out=mean_all[:, t:t + 1])
                    # xs = X * sqrt(exp(w_log_var)); written bf16 so the
                    # scalar engine can square-accumulate at 2x. Every 5th
                    # tile runs on the otherwise-idle gpsimd engine (slower
                    # per-op, but off the critical DVE path).
                    eng = nc.gpsimd if t % 5 == 4 else nc.vector
                    eng.tensor_mul(xs[:], xt[:], qb[:])
                    # var_t = rowsum(xs^2)
                    sq = xsp.tile([P, D], XSDT, tag="sq")
                    nc.scalar.activation(sq[:], xs[:], AF.Square,
                                         accum_out=var_all[:, t:t + 1])
                    if t % G == G - 1:
                        g = t // G
                        dve_std(slice(g * G, (g + 1) * G))
                        for tt in range(g * G, (g + 1) * G):
                            tile_out(tt)

            if reps == 1:
                body()
            else:
                with tc.For_i(0, reps, 1):
                    body()

    nc.compile()
    return nc


def _get_nc():
    global _cached_nc
    if _cached_nc is None:
        _cached_nc = build_program()
    return _cached_nc


def make_host_inputs(X, z, w_mu, w_log_var):
    """Host-side prep: exp of the [D] vector + broadcast of w_mu."""
    X = np.ascontiguousarray(np.asarray(X, dtype=np.float32))
    z = np.asarray(z, dtype=np.float32)
    w_mu = np.asarray(w_mu, dtype=np.float32)
    w_log_var = np.asarray(w_log_var, dtype=np.float32)
    sqew = np.exp(0.5 * w_log_var).astype(np.float32)  # sqrt(exp(w_log_var))
    wb = np.ascontiguousarray(np.broadcast_to(w_mu, (P, D)))
    qv = np.ascontiguousarray(sqew.reshape(1, D))
    zv = np.ascontiguousarray(z.reshape(1, S))
    in_maps = [
        {"x": X[k * NSHARD:(k + 1) * NSHARD], "wb": wb, "qv": qv, "zv": zv}
        for k in range(NCORES)
    ]
    return in_maps


def kernel(X, z, w_mu, w_log_var):
    global last_result
    nc = _get_nc()
    in_maps = make_host_inputs(X, z, w_mu, w_log_var)
    trace = bool(int(os.environ.get("KTRACE", "0")))
    res = run_bass_kernel_spmd(nc, in_maps, list(range(NCORES)), trace=trace)
    last_result = res
    Y = np.concatenate([r["y"] for r in res.results], axis=0)
    act = np.concatenate([r["act"] for r in res.results], axis=0)
    return (Y, act)
